# revision 1
# baseline (speedup 1.0000x reference)
"""Trainium2 Bass kernel for per-pixel local convolution (LocalConvolution).

Reference semantics (fp32):
    out[n, g*8+j, ho, wo] = sum_{i,jj in 5x5} x[n, g*8+j, ho+i-2, wo+jj-2]
                                             * w[n, j, i*5+jj, ho*128+wo]
with zero padding, N=4, C=64, H=W=128, CW=8, K=5.

Sharding: 8 cores = (batch n in 4) x (H-half in 2). Each core computes a
[64, 64, 128] output slab from a host-padded input slab [64, 69, 132] and a
weight slab [8, 25, 8192].

On-core layout: 128 partitions = (weight-channel j in 8) x (4-row block pc in
16); looping over the 8 channel groups g reuses one weight residency (no
replication). Pixels are streamed as 528 "pseudo-pixels" per g and partition
(4 rows x 132 cols incl. right-pad); pad columns carry zero weights so they
contribute nothing, which keeps every tap stream a contiguous slice.

Compute paths:
  LC_PHASE=1  stock two-pass: 25x tensor_mul + strided tensor_reduce over taps
  LC_PHASE=2  (default) fused single-pass: custom DVE op out = cumsum(in0*in1)
              along the free stream (1 MAC/cycle/lane, the fp32 2-read-port
              bound); per-pixel sums are recovered on GPSIMD (overlapped) by
              sampling the prefix at tap boundaries and differencing.
"""

import os

import numpy as np

try:
    import concourse.bass as bass
except ImportError:  # fresh grading dir: concourse lives in the container image
    import sys

    for p in ("/opt/trn_rl_repo", "/root/.axon_site/_ro/trn_rl_repo"):
        if p not in sys.path:
            sys.path.insert(0, p)
    import concourse.bass as bass

import concourse.mybir as mybir
from concourse import tile
from concourse.bass_utils import run_bass_kernel_spmd

# ---------------------------------------------------------------------------
# Workaround: this walrus build accepts only ONE sync-wait per instruction,
# but Tile's semaphore assignment freely fuses several. Post-pass: peel extra
# waits off onto preceding same-engine NOPs (engine streams execute in order,
# so the NOPs block the engine until every condition holds).
# ---------------------------------------------------------------------------


def _split_multi_waits(nc):
    n_split = 0
    for fn in nc.m.functions:
        for bb in fn.blocks:
            new_insts = []
            for inst in bb.instructions:
                si = inst.sync_info
                if si is not None and len(si.on_wait) > 1:
                    waits = list(si.on_wait)
                    for k, w in enumerate(waits[:-1]):
                        n_split += 1
                        new_insts.append(
                            mybir.InstNoOp(
                                name=f"{inst.name}_w{k}",
                                engine=inst.engine,
                                sync_info=mybir.SyncInfo(
                                    on_wait=[w], on_update=[]
                                ),
                                bass_nofuse=True,
                            )
                        )
                    inst.sync_info = mybir.SyncInfo(
                        on_wait=[waits[-1]], on_update=list(si.on_update)
                    )
                new_insts.append(inst)
            bb.instructions = new_insts
    return n_split

# ---------------------------------------------------------------------------
# Custom DVE op: fused multiply + running prefix sum along the free stream.
#   out[p, t] = sum_{t' <= t} in0[p, t'] * in1[p, t']      (fp32 accumulator)
# ---------------------------------------------------------------------------


def _register_mac_prefix():
    from concourse import dve_ops
    from concourse.dve_spec import (
        AluOp,
        Spec,
        Src0,
        Src1,
        _has_src1,
        lower,
        scan,
    )
    from concourse.dve_table_gen import dve_ver_for
    from concourse.dve_uop import DveOpSpec

    name = "MAC_PREFIX_ANT"
    if name in dve_ops._SUB_OPCODE_FOR_NAME:
        return next(op for op in dve_ops.OPS if op.name == name)

    def _ref(in0, in1, s0, s1, imm2):
        prod = in0.astype(np.float32) * in1.astype(np.float32)
        flat = prod.reshape(prod.shape[0], -1)
        return np.cumsum(flat, axis=1).reshape(prod.shape)

    spec = Spec(body=scan(AluOp.ADD, Src0 * Src1), reference=_ref)
    row = dve_ops._CUSTOM_DVE_ROW_BASE + len(dve_ops.OPS)
    assert row < 0x20
    shas = {}
    for ver in {dve_ver_for("TRN2"), "v3", "v4"}:
        compiled = DveOpSpec(
            name=name, opcode=row, uops=lower(spec, ver=ver), rd1_en=_has_src1(spec)
        )
        shas[ver] = compiled.sha(ver)
    op = dve_ops.DveOp(name, spec, subdim=False, uops_sha=shas)
    dve_ops.OPS.append(op)
    dve_ops.CUSTOM_DVE_SPECS[name] = spec
    dve_ops._SUB_OPCODE_FOR_NAME[name] = row
    return op


# ---------------------------------------------------------------------------
# Problem constants
# ---------------------------------------------------------------------------
N, C, H, W = 4, 64, 128, 128
K, PAD, CW = 5, 2, 8
HO, WO = 128, 128
RH = 64  # output rows per core
WP = W + 2 * PAD  # 132: padded row length
HP = RH + K  # 69: padded rows per core slab (64 + 4 halo + 1 guard)
NJ, NPC, RB = 8, 16, 4  # partition = j*16 + pc; RB output rows per pc
Q = RB * WP  # 528 pseudo-pixels per partition per group
F32 = mybir.dt.float32
X = mybir.AxisListType.X
ADD = mybir.AluOpType.add

PHASE = int(os.environ.get("LC_PHASE", "2"))


def _build_program(phase, repeat=1):
    nc = bass.Bass()
    xpad_d = nc.declare_dram_parameter("xpad", [C, HP, WP], F32, isOutput=False)
    w_d = nc.declare_dram_parameter("w", [CW, K * K, RH * WO], F32, isOutput=False)
    out_d = nc.declare_dram_parameter("out", [C, RH, WO], F32, isOutput=True)

    if phase == 2:
        mac_prefix = _register_mac_prefix()

    xpad_a = xpad_d[:]
    w_a = w_d[:]
    out_a = out_d[:]

    with tile.TileContext(nc) as tc:
        with (
            tc.tile_pool(name="wpool", bufs=1) as wpool,
            tc.tile_pool(name="xpool", bufs=3) as xpool,
            tc.tile_pool(name="big", bufs=2) as bigpool,
            tc.tile_pool(name="tpool", bufs=2) as tpool,
            tc.tile_pool(name="ogpool", bufs=3) as ogpool,
        ):
            # ---- weight residency: 5 tiles (one per kernel row i), layout
            # [p=(j,pc), jj, q] with q = 4x132 pseudo-pixels, cols 128..131
            # zeroed so pad pixels multiply to exactly 0.
            w_tiles = []
            for i in range(K):
                wt = wpool.tile([128, K * Q], F32, tag=f"w{i}")
                wa = wt[:]
                # zero the 4 pad columns of each (jj, row) stripe
                pad_ap = wa.__replace__(
                    ap=[wa.ap[0], [Q, K], [WP, RB], [1, WP - WO]],
                    offset=wa.offset + WO,
                )
                nc.gpsimd.memset(pad_ap, 0.0)
                for jj in range(K):
                    kk = i * K + jj
                    dst = wa.__replace__(
                        ap=[wa.ap[0], [WP, RB], [1, WO]],
                        offset=wa.offset + jj * Q,
                    )
                    src = w_a.__replace__(
                        ap=[[K * K * RH * WO, NJ], [RB * WO, NPC], [1, RB * WO]],
                        offset=kk * RH * WO,
                    )
                    nc.sync.dma_start(dst, src)
                w_tiles.append(wt)

            for g in range(repeat * C // CW):
                g = g % (C // CW)
                # ---- input slab for this channel group: partition (j, pc)
                # holds 9 padded rows x 132 cols of channel c = g*8+j.
                xg = xpool.tile([128, (RB + K) * WP], F32, tag="xg")
                xa = xg[:]
                nc.sync.dma_start(
                    xa,
                    xpad_a.__replace__(
                        ap=[[HP * WP, NJ], [RB * WP, NPC], [1, (RB + K) * WP]],
                        offset=g * CW * HP * WP,
                    ),
                )

                og = ogpool.tile([128, Q], F32, tag="og")
                if phase == 1:
                    prod = bigpool.tile([128, K * K * Q], F32, tag="prod")
                    for kk in range(K * K):
                        i, jj = divmod(kk, K)
                        nc.vector.tensor_mul(
                            prod[:, kk * Q : (kk + 1) * Q],
                            xg[:, i * WP + jj : i * WP + jj + Q],
                            w_tiles[i][:, jj * Q : (jj + 1) * Q],
                        )
                    pa = prod[:]
                    nc.vector.tensor_reduce(
                        og[:, :],
                        pa.__replace__(ap=[pa.ap[0], [1, Q], [Q, K * K]]),
                        axis=X,
                        op=ADD,
                    )
                else:
                    # T[q] accumulates each kernel row's prefix sampled at its
                    # tap boundary (jj=4)
                    t = tpool.tile([128, Q], F32, tag="t")
                    for i in range(K):
                        pre = bigpool.tile([128, K * Q], F32, tag="pre")
                        prea = pre[:]
                        in0 = xa.__replace__(
                            ap=[xa.ap[0], [1, Q], [1, K]],
                            offset=xa.offset + i * WP,
                        )
                        wa = w_tiles[i][:]
                        in1 = wa.__replace__(
                            ap=[wa.ap[0], [1, Q], [Q, K]], offset=wa.offset
                        )
                        nc.vector._custom_dve(
                            mac_prefix, out=prea, in0=in0, in1=in1
                        )
                        boundary = prea.__replace__(
                            ap=[prea.ap[0], [K, Q]], offset=prea.offset + (K - 1)
                        )
                        # boundary extraction runs on GPSIMD, overlapping the
                        # next scan on the vector engine
                        if i == 0:
                            nc.gpsimd.tensor_copy(t[:, :], boundary)
                        else:
                            nc.gpsimd.tensor_add(t[:, :], t[:, :], boundary)
                    # per-pixel sums: out[q] = T[q] - T[q-1] (garbage pixels
                    # contribute zero, so row-crossing diffs stay exact)
                    nc.gpsimd.tensor_sub(og[:, 1:Q], t[:, 1:Q], t[:, 0 : Q - 1])
                    nc.gpsimd.tensor_copy(og[:, 0:1], t[:, 0:1])

                oga = og[:]
                nc.sync.dma_start(
                    out_a.__replace__(
                        ap=[[RH * WO, NJ], [RB * WO, NPC], [WO, RB], [1, WO]],
                        offset=g * CW * RH * WO,
                    ),
                    oga.__replace__(
                        ap=[oga.ap[0], [WP, RB], [1, WO]], offset=oga.offset
                    ),
                )
    # raw Bass skips the ISA-subclass byte encoding pass that Bacc.compile
    # runs; without it the NEFF compiler sees empty .instr -> "ISA wrong length"
    mybir.codegen_inst_isa_subclasses(nc)
    _split_multi_waits(nc)
    return nc


def _shard_inputs(input, weight):
    input = np.asarray(input, dtype=np.float32)
    weight = np.asarray(weight, dtype=np.float32)
    in_maps = []
    for n in range(N):
        xp = np.pad(input[n], ((0, 0), (PAD, PAD + 1), (PAD, PAD)))  # [64,133,132]
        for half in range(2):
            r0 = RH * half
            in_maps.append(
                {
                    "xpad": np.ascontiguousarray(xp[:, r0 : r0 + HP, :]),
                    "w": np.ascontiguousarray(
                        weight[n, :, :, r0 * WO : (r0 + RH) * WO]
                    ),
                }
            )
    return in_maps


def kernel(input, weight):
    nc = _build_program(PHASE)
    in_maps = _shard_inputs(input, weight)
    res = run_bass_kernel_spmd(nc, in_maps, list(range(8)))
    out = np.empty((N, C, HO, WO), dtype=np.float32)
    for k in range(8):
        n, half = divmod(k, 2)
        out[n, :, RH * half : RH * (half + 1), :] = res.results[k]["out"]
    return out



# revision 8
# speedup vs baseline: 1.0858x; 1.0858x over previous
"""Trainium2 Bass kernel for per-pixel local convolution (LocalConvolution).

Reference semantics (fp32):
    out[n, g*8+j, ho, wo] = sum_{i,jj in 5x5} x[n, g*8+j, ho+i-2, wo+jj-2]
                                             * w[n, j, i*5+jj, ho*128+wo]
with zero padding, N=4, C=64, H=W=128, CW=8, K=5.

Sharding: 8 cores = (batch n in 4) x (H-half in 2). Each core computes a
[64, 64, 128] output slab from a host-padded input slab [64, 69, 132] and a
weight slab [8, 25, 8192].

On-core layout: 128 partitions = (weight-channel j in 8) x (4-row block pc in
16); looping over the 8 channel groups g reuses one weight residency (no
replication). Pixels are streamed as 528 "pseudo-pixels" per g and partition
(4 rows x 132 cols incl. right-pad); pad columns carry zero weights so they
contribute nothing, which keeps every tap stream a contiguous slice.

Compute paths:
  LC_PHASE=1  stock two-pass: 25x tensor_mul + strided tensor_reduce over taps
  LC_PHASE=2  (default) fused single-pass: custom DVE op out = cumsum(in0*in1)
              along the free stream (1 MAC/cycle/lane, the fp32 2-read-port
              bound); per-pixel sums are recovered on GPSIMD (overlapped) by
              sampling the prefix at tap boundaries and differencing.
"""

import os

import numpy as np

try:
    import concourse.bass as bass
except ImportError:  # fresh grading dir: concourse lives in the container image
    import sys

    for p in ("/opt/trn_rl_repo", "/root/.axon_site/_ro/trn_rl_repo"):
        if p not in sys.path:
            sys.path.insert(0, p)
    import concourse.bass as bass

import concourse.mybir as mybir
from concourse import tile
from concourse.bass_utils import run_bass_kernel_spmd

# ---------------------------------------------------------------------------
# Workaround: this walrus build accepts only ONE sync-wait per instruction,
# but Tile's semaphore assignment freely fuses several. Post-pass: peel extra
# waits off onto preceding same-engine NOPs (engine streams execute in order,
# so the NOPs block the engine until every condition holds).
# ---------------------------------------------------------------------------


def _split_multi_waits(nc):
    n_split = 0
    for fn in nc.m.functions:
        for bb in fn.blocks:
            new_insts = []
            for inst in bb.instructions:
                si = inst.sync_info
                if si is not None and len(si.on_wait) > 1:
                    waits = list(si.on_wait)
                    for k, w in enumerate(waits[:-1]):
                        n_split += 1
                        new_insts.append(
                            mybir.InstNoOp(
                                name=f"{inst.name}_w{k}",
                                engine=inst.engine,
                                sync_info=mybir.SyncInfo(
                                    on_wait=[w], on_update=[]
                                ),
                                bass_nofuse=True,
                            )
                        )
                    inst.sync_info = mybir.SyncInfo(
                        on_wait=[waits[-1]], on_update=list(si.on_update)
                    )
                new_insts.append(inst)
            bb.instructions = new_insts
    return n_split

# ---------------------------------------------------------------------------
# Custom DVE op: fused multiply + running prefix sum along the free stream.
#   out[p, t] = sum_{t' <= t} in0[p, t'] * in1[p, t']      (fp32 accumulator)
# ---------------------------------------------------------------------------


def _register_mac_prefix():
    from concourse import dve_ops
    from concourse.dve_spec import (
        AluOp,
        Spec,
        Src0,
        Src1,
        _has_src1,
        lower,
        scan,
    )
    from concourse.dve_table_gen import dve_ver_for
    from concourse.dve_uop import DveOpSpec

    name = "MAC_PREFIX_ANT"
    if name in dve_ops._SUB_OPCODE_FOR_NAME:
        return next(op for op in dve_ops.OPS if op.name == name)

    def _ref(in0, in1, s0, s1, imm2):
        prod = in0.astype(np.float32) * in1.astype(np.float32)
        flat = prod.reshape(prod.shape[0], -1)
        return np.cumsum(flat, axis=1).reshape(prod.shape)

    spec = Spec(body=scan(AluOp.ADD, Src0 * Src1), reference=_ref)
    row = dve_ops._CUSTOM_DVE_ROW_BASE + len(dve_ops.OPS)
    assert row < 0x20
    shas = {}
    for ver in {dve_ver_for("TRN2"), "v3", "v4"}:
        compiled = DveOpSpec(
            name=name, opcode=row, uops=lower(spec, ver=ver), rd1_en=_has_src1(spec)
        )
        shas[ver] = compiled.sha(ver)
    op = dve_ops.DveOp(name, spec, subdim=False, uops_sha=shas)
    dve_ops.OPS.append(op)
    dve_ops.CUSTOM_DVE_SPECS[name] = spec
    dve_ops._SUB_OPCODE_FOR_NAME[name] = row
    return op


# ---------------------------------------------------------------------------
# Problem constants
# ---------------------------------------------------------------------------
N, C, H, W = 4, 64, 128, 128
K, PAD, CW = 5, 2, 8
HO, WO = 128, 128
RH = 64  # output rows per core
WP = W + 2 * PAD  # 132: padded row length
HP = RH + K  # 69: padded rows per core slab (64 + 4 halo + 1 guard)
NJ, NPC, RB = 8, 16, 4  # partition = j*16 + pc; RB output rows per pc
Q = RB * WP  # 528 pseudo-pixels per partition per group
F32 = mybir.dt.float32
X = mybir.AxisListType.X
ADD = mybir.AluOpType.add

PHASE = int(os.environ.get("LC_PHASE", "2"))


def _build_program(phase, repeat=1):
    nc = bass.Bass()
    xpad_d = nc.declare_dram_parameter("xpad", [C, HP, WP], F32, isOutput=False)
    w_d = nc.declare_dram_parameter("w", [CW, K * K, RH * WO], F32, isOutput=False)
    out_d = nc.declare_dram_parameter("out", [C, RH, WO], F32, isOutput=True)

    if phase == 2:
        mac_prefix = _register_mac_prefix()

    xpad_a = xpad_d[:]
    w_a = w_d[:]
    out_a = out_d[:]

    with tile.TileContext(nc) as tc:
        with (
            tc.tile_pool(name="wpool", bufs=1) as wpool,
            tc.tile_pool(name="xpool", bufs=3) as xpool,
            tc.tile_pool(name="big", bufs=2) as bigpool,
            tc.tile_pool(name="tpool", bufs=2) as tpool,
            tc.tile_pool(name="ogpool", bufs=3) as ogpool,
        ):
            # ---- weight residency: 5 tiles (one per kernel row i), layout
            # [p=(j,pc), jj, q] with q = 4x132 pseudo-pixels, cols 128..131
            # zeroed so pad pixels multiply to exactly 0.
            w_tiles = []
            for i in range(K):
                wt = wpool.tile([128, K * Q], F32, tag=f"w{i}")
                wa = wt[:]
                # zero the 4 pad columns of each (jj, row) stripe
                pad_ap = wa.__replace__(
                    ap=[wa.ap[0], [Q, K], [WP, RB], [1, WP - WO]],
                    offset=wa.offset + WO,
                )
                nc.gpsimd.memset(pad_ap, 0.0)
                for jj in range(K):
                    kk = i * K + jj
                    dst = wa.__replace__(
                        ap=[wa.ap[0], [WP, RB], [1, WO]],
                        offset=wa.offset + jj * Q,
                    )
                    src = w_a.__replace__(
                        ap=[[K * K * RH * WO, NJ], [RB * WO, NPC], [1, RB * WO]],
                        offset=kk * RH * WO,
                    )
                    nc.sync.dma_start(dst, src)
                w_tiles.append(wt)
                if i == 0:
                    # prefetch group 0's input slab right behind the first
                    # weight tile so compute starts ~18us earlier (the
                    # remaining weight-tile DMAs overlap the first scans)
                    xg0 = xpool.tile([128, (RB + K) * WP], F32, tag="xg")
                    nc.sync.dma_start(
                        xg0[:],
                        xpad_a.__replace__(
                            ap=[
                                [HP * WP, NJ],
                                [RB * WP, NPC],
                                [1, (RB + K) * WP],
                            ],
                            offset=0,
                        ),
                    )

            for g in range(repeat * C // CW):
                g = g % (C // CW)
                # ---- input slab for this channel group: partition (j, pc)
                # holds 9 padded rows x 132 cols of channel c = g*8+j.
                if g == 0:
                    xg = xg0
                    xa = xg[:]
                else:
                    xg = xpool.tile([128, (RB + K) * WP], F32, tag="xg")
                    xa = xg[:]
                    nc.sync.dma_start(
                        xa,
                        xpad_a.__replace__(
                            ap=[
                                [HP * WP, NJ],
                                [RB * WP, NPC],
                                [1, (RB + K) * WP],
                            ],
                            offset=g * CW * HP * WP,
                        ),
                    )

                og = ogpool.tile([128, Q], F32, tag="og")
                if phase == 1:
                    prod = bigpool.tile([128, K * K * Q], F32, tag="prod")
                    for kk in range(K * K):
                        i, jj = divmod(kk, K)
                        nc.vector.tensor_mul(
                            prod[:, kk * Q : (kk + 1) * Q],
                            xg[:, i * WP + jj : i * WP + jj + Q],
                            w_tiles[i][:, jj * Q : (jj + 1) * Q],
                        )
                    pa = prod[:]
                    nc.vector.tensor_reduce(
                        og[:, :],
                        pa.__replace__(ap=[pa.ap[0], [1, Q], [Q, K * K]]),
                        axis=X,
                        op=ADD,
                    )
                else:
                    # T[q] accumulates each kernel row's prefix sampled at its
                    # tap boundary (jj=4)
                    t = tpool.tile([128, Q], F32, tag="t")
                    # the last group's boundary extraction runs on the DVE
                    # (idle once its scans finish) instead of Pool, cutting
                    # the Pool-only tail after the final scan
                    beng = nc.vector if g == C // CW - 1 else nc.gpsimd
                    for i in range(K):
                        pre = bigpool.tile([128, K * Q], F32, tag="pre")
                        prea = pre[:]
                        in0 = xa.__replace__(
                            ap=[xa.ap[0], [1, Q], [1, K]],
                            offset=xa.offset + i * WP,
                        )
                        wa = w_tiles[i][:]
                        in1 = wa.__replace__(
                            ap=[wa.ap[0], [1, Q], [Q, K]], offset=wa.offset
                        )
                        nc.vector._custom_dve(
                            mac_prefix, out=prea, in0=in0, in1=in1
                        )
                        boundary = prea.__replace__(
                            ap=[prea.ap[0], [K, Q]], offset=prea.offset + (K - 1)
                        )
                        # boundary extraction runs on GPSIMD, overlapping the
                        # next scan on the vector engine
                        if i == 0:
                            nc.gpsimd.tensor_copy(t[:, :], boundary)
                        else:
                            nc.gpsimd.tensor_add(t[:, :], t[:, :], boundary)
                    # per-pixel sums: out[q] = T[q] - T[q-1] (garbage pixels
                    # contribute zero, so row-crossing diffs stay exact)
                    beng.tensor_sub(og[:, 1:Q], t[:, 1:Q], t[:, 0 : Q - 1])
                    beng.tensor_copy(og[:, 0:1], t[:, 0:1])

                oga = og[:]
                nc.sync.dma_start(
                    out_a.__replace__(
                        ap=[[RH * WO, NJ], [RB * WO, NPC], [WO, RB], [1, WO]],
                        offset=g * CW * RH * WO,
                    ),
                    oga.__replace__(
                        ap=[oga.ap[0], [WP, RB], [1, WO]], offset=oga.offset
                    ),
                )
    # raw Bass skips the ISA-subclass byte encoding pass that Bacc.compile
    # runs; without it the NEFF compiler sees empty .instr -> "ISA wrong length"
    mybir.codegen_inst_isa_subclasses(nc)
    _split_multi_waits(nc)
    return nc


def _shard_inputs(input, weight):
    input = np.asarray(input, dtype=np.float32)
    weight = np.asarray(weight, dtype=np.float32)
    in_maps = []
    for n in range(N):
        xp = np.pad(input[n], ((0, 0), (PAD, PAD + 1), (PAD, PAD)))  # [64,133,132]
        for half in range(2):
            r0 = RH * half
            in_maps.append(
                {
                    "xpad": np.ascontiguousarray(xp[:, r0 : r0 + HP, :]),
                    "w": np.ascontiguousarray(
                        weight[n, :, :, r0 * WO : (r0 + RH) * WO]
                    ),
                }
            )
    return in_maps


def kernel(input, weight):
    nc = _build_program(PHASE)
    in_maps = _shard_inputs(input, weight)
    res = run_bass_kernel_spmd(nc, in_maps, list(range(8)))
    out = np.empty((N, C, HO, WO), dtype=np.float32)
    for k in range(8):
        n, half = divmod(k, 2)
        out[n, :, RH * half : RH * (half + 1), :] = res.results[k]["out"]
    return out



# revision 11
# speedup vs baseline: 1.6765x; 1.5440x over previous
"""Trainium2 Bass kernel for per-pixel local convolution (LocalConvolution).

Reference semantics (fp32):
    out[n, g*8+j, ho, wo] = sum_{i,jj in 5x5} x[n, g*8+j, ho+i-2, wo+jj-2]
                                             * w[n, j, i*5+jj, ho*128+wo]
with zero padding, N=4, C=64, H=W=128, CW=8, K=5.

Sharding: 8 cores = (batch n in 4) x (H-half in 2). Each core computes a
[64, 64, 128] output slab from a host-padded input slab [64, 69, 132] and a
weight slab [8, 25, 8192].

On-core layout: 128 partitions = (weight-channel j in 8) x (4-row block pc in
16); looping over the 8 channel groups g reuses one weight residency (no
replication). Pixels are streamed as 528 "pseudo-pixels" per g and partition
(4 rows x 132 cols incl. right-pad); pad columns carry zero weights so they
contribute nothing, which keeps every tap stream a contiguous slice.

Compute paths:
  LC_PHASE=1  stock two-pass: 25x tensor_mul + strided tensor_reduce over taps
  LC_PHASE=2  (default) fused single-pass: custom DVE op out = cumsum(in0*in1)
              along the free stream (1 MAC/cycle/lane, the fp32 2-read-port
              bound); per-pixel sums are recovered on GPSIMD (overlapped) by
              sampling the prefix at tap boundaries and differencing.
"""

import os

import numpy as np

try:
    import concourse.bass as bass
except ImportError:  # fresh grading dir: concourse lives in the container image
    import sys

    for p in ("/opt/trn_rl_repo", "/root/.axon_site/_ro/trn_rl_repo"):
        if p not in sys.path:
            sys.path.insert(0, p)
    import concourse.bass as bass

import concourse.mybir as mybir
from concourse import tile
from concourse.bass_utils import run_bass_kernel_spmd

# ---------------------------------------------------------------------------
# Workaround: this walrus build accepts only ONE sync-wait per instruction,
# but Tile's semaphore assignment freely fuses several. Post-pass: peel extra
# waits off onto preceding same-engine NOPs (engine streams execute in order,
# so the NOPs block the engine until every condition holds).
# ---------------------------------------------------------------------------


def _split_multi_waits(nc):
    n_split = 0
    for fn in nc.m.functions:
        for bb in fn.blocks:
            new_insts = []
            for inst in bb.instructions:
                si = inst.sync_info
                if si is not None and len(si.on_wait) > 1:
                    waits = list(si.on_wait)
                    for k, w in enumerate(waits[:-1]):
                        n_split += 1
                        new_insts.append(
                            mybir.InstNoOp(
                                name=f"{inst.name}_w{k}",
                                engine=inst.engine,
                                sync_info=mybir.SyncInfo(
                                    on_wait=[w], on_update=[]
                                ),
                                bass_nofuse=True,
                            )
                        )
                    inst.sync_info = mybir.SyncInfo(
                        on_wait=[waits[-1]], on_update=list(si.on_update)
                    )
                new_insts.append(inst)
            bb.instructions = new_insts
    return n_split

# ---------------------------------------------------------------------------
# Custom DVE op: fused multiply + running prefix sum along the free stream.
#   out[p, t] = sum_{t' <= t} in0[p, t'] * in1[p, t']      (fp32 accumulator)
# ---------------------------------------------------------------------------


def _register_mac_prefix():
    from concourse import dve_ops
    from concourse.dve_spec import (
        AluOp,
        Spec,
        Src0,
        Src1,
        _has_src1,
        lower,
        scan,
    )
    from concourse.dve_table_gen import dve_ver_for
    from concourse.dve_uop import DveOpSpec

    name = "MAC_PREFIX_ANT"
    if name in dve_ops._SUB_OPCODE_FOR_NAME:
        return next(op for op in dve_ops.OPS if op.name == name)

    def _ref(in0, in1, s0, s1, imm2):
        prod = in0.astype(np.float32) * in1.astype(np.float32)
        flat = prod.reshape(prod.shape[0], -1)
        return np.cumsum(flat, axis=1).reshape(prod.shape)

    spec = Spec(body=scan(AluOp.ADD, Src0 * Src1), reference=_ref)
    row = dve_ops._CUSTOM_DVE_ROW_BASE + len(dve_ops.OPS)
    assert row < 0x20
    shas = {}
    for ver in {dve_ver_for("TRN2"), "v3", "v4"}:
        compiled = DveOpSpec(
            name=name, opcode=row, uops=lower(spec, ver=ver), rd1_en=_has_src1(spec)
        )
        shas[ver] = compiled.sha(ver)
    op = dve_ops.DveOp(name, spec, subdim=False, uops_sha=shas)
    dve_ops.OPS.append(op)
    dve_ops.CUSTOM_DVE_SPECS[name] = spec
    dve_ops._SUB_OPCODE_FOR_NAME[name] = row
    return op


# ---------------------------------------------------------------------------
# Problem constants
# ---------------------------------------------------------------------------
N, C, H, W = 4, 64, 128, 128
K, PAD, CW = 5, 2, 8
HO, WO = 128, 128
RH = 64  # output rows per core
WP = W + 2 * PAD  # 132: padded row length
HP = RH + K  # 69: padded rows per core slab (64 + 4 halo + 1 guard)
NJ, NPC, RB = 8, 16, 4  # partition = j*16 + pc; RB output rows per pc
Q = RB * WP  # 528 pseudo-pixels per partition per group
F32 = mybir.dt.float32
X = mybir.AxisListType.X
ADD = mybir.AluOpType.add

PHASE = int(os.environ.get("LC_PHASE", "2"))
# LC_W16=1: weights travel as fp16 in the exact SBUF tile layout (one
# contiguous DMA per kernel-row tile, pad columns pre-zeroed on the host).
# Halves weight DMA time and removes the sub-512B-run descriptor penalty,
# shortening the startup ramp; the scan consumes fp16 in1 directly.
W16 = int(os.environ.get("LC_W16", "0"))
F16 = mybir.dt.float16


def _build_program(phase, repeat=1):
    nc = bass.Bass()
    xpad_d = nc.declare_dram_parameter("xpad", [C, HP, WP], F32, isOutput=False)
    if W16:
        w_d = nc.declare_dram_parameter("w16", [K, 128, K * Q], F16, isOutput=False)
    else:
        w_d = nc.declare_dram_parameter(
            "w", [CW, K * K, RH * WO], F32, isOutput=False
        )
    out_d = nc.declare_dram_parameter("out", [C, RH, WO], F32, isOutput=True)

    if phase == 2:
        mac_prefix = _register_mac_prefix()

    xpad_a = xpad_d[:]
    w_a = w_d[:]
    out_a = out_d[:]

    with tile.TileContext(nc) as tc:
        with (
            tc.tile_pool(name="wpool", bufs=1) as wpool,
            tc.tile_pool(name="xpool", bufs=3) as xpool,
            tc.tile_pool(name="big", bufs=2) as bigpool,
            tc.tile_pool(name="tpool", bufs=2) as tpool,
            tc.tile_pool(name="ogpool", bufs=3) as ogpool,
        ):
            # ---- weight residency: 5 tiles (one per kernel row i), layout
            # [p=(j,pc), jj, q] with q = 4x132 pseudo-pixels, cols 128..131
            # zeroed so pad pixels multiply to exactly 0.
            w_tiles = []
            for i in range(K):
                wt = wpool.tile([128, K * Q], F32, tag=f"w{i}")
                wa = wt[:]
                # zero the 4 pad columns of each (jj, row) stripe
                pad_ap = wa.__replace__(
                    ap=[wa.ap[0], [Q, K], [WP, RB], [1, WP - WO]],
                    offset=wa.offset + WO,
                )
                nc.gpsimd.memset(pad_ap, 0.0)
                for jj in range(K):
                    kk = i * K + jj
                    dst = wa.__replace__(
                        ap=[wa.ap[0], [WP, RB], [1, WO]],
                        offset=wa.offset + jj * Q,
                    )
                    src = w_a.__replace__(
                        ap=[[K * K * RH * WO, NJ], [RB * WO, NPC], [1, RB * WO]],
                        offset=kk * RH * WO,
                    )
                    nc.sync.dma_start(dst, src)
                w_tiles.append(wt)
                if i == 0:
                    # prefetch group 0's input slab right behind the first
                    # weight tile so compute starts ~18us earlier (the
                    # remaining weight-tile DMAs overlap the first scans)
                    xg0 = xpool.tile([128, (RB + K) * WP], F32, tag="xg")
                    nc.sync.dma_start(
                        xg0[:],
                        xpad_a.__replace__(
                            ap=[
                                [HP * WP, NJ],
                                [RB * WP, NPC],
                                [1, (RB + K) * WP],
                            ],
                            offset=0,
                        ),
                    )

            for g in range(repeat * C // CW):
                g = g % (C // CW)
                # ---- input slab for this channel group: partition (j, pc)
                # holds 9 padded rows x 132 cols of channel c = g*8+j.
                if g == 0:
                    xg = xg0
                    xa = xg[:]
                else:
                    xg = xpool.tile([128, (RB + K) * WP], F32, tag="xg")
                    xa = xg[:]
                    nc.sync.dma_start(
                        xa,
                        xpad_a.__replace__(
                            ap=[
                                [HP * WP, NJ],
                                [RB * WP, NPC],
                                [1, (RB + K) * WP],
                            ],
                            offset=g * CW * HP * WP,
                        ),
                    )

                og = ogpool.tile([128, Q], F32, tag="og")
                if phase == 1:
                    prod = bigpool.tile([128, K * K * Q], F32, tag="prod")
                    for kk in range(K * K):
                        i, jj = divmod(kk, K)
                        nc.vector.tensor_mul(
                            prod[:, kk * Q : (kk + 1) * Q],
                            xg[:, i * WP + jj : i * WP + jj + Q],
                            w_tiles[i][:, jj * Q : (jj + 1) * Q],
                        )
                    pa = prod[:]
                    nc.vector.tensor_reduce(
                        og[:, :],
                        pa.__replace__(ap=[pa.ap[0], [1, Q], [Q, K * K]]),
                        axis=X,
                        op=ADD,
                    )
                else:
                    # T[q] accumulates each kernel row's prefix sampled at its
                    # tap boundary (jj=4)
                    t = tpool.tile([128, Q], F32, tag="t")



                    beng = nc.vector if g == C // CW - 1 else nc.gpsimd
                    for i in range(K):
                        pre = bigpool.tile([128, K * Q], F32, tag="pre")
                        prea = pre[:]
                        in0 = xa.__replace__(
                            ap=[xa.ap[0], [1, Q], [1, K]],
                            offset=xa.offset + i * WP,
                        )
                        wa = w_tiles[i][:]
                        in1 = wa.__replace__(
                            ap=[wa.ap[0], [1, Q], [Q, K]], offset=wa.offset
                        )
                        nc.vector._custom_dve(
                            mac_prefix, out=prea, in0=in0, in1=in1
                        )
                        boundary = prea.__replace__(
                            ap=[prea.ap[0], [K, Q]], offset=prea.offset + (K - 1)
                        )
                        # boundary extraction runs on GPSIMD, overlapping the
                        # next scan on the vector engine
                        if i == 0:
                            nc.gpsimd.tensor_copy(t[:, :], boundary)
                        else:
                            nc.gpsimd.tensor_add(t[:, :], t[:, :], boundary)
                    # per-pixel sums: out[q] = T[q] - T[q-1] (garbage pixels
                    # contribute zero, so row-crossing diffs stay exact)
                    nc.gpsimd.tensor_sub(og[:, 1:Q], t[:, 1:Q], t[:, 0 : Q - 1])
                    nc.gpsimd.tensor_copy(og[:, 0:1], t[:, 0:1])

                oga = og[:]
                nc.sync.dma_start(
                    out_a.__replace__(
                        ap=[[RH * WO, NJ], [RB * WO, NPC], [WO, RB], [1, WO]],
                        offset=g * CW * RH * WO,
                    ),
                    oga.__replace__(
                        ap=[oga.ap[0], [WP, RB], [1, WO]], offset=oga.offset
                    ),
                )
    # raw Bass skips the ISA-subclass byte encoding pass that Bacc.compile
    # runs; without it the NEFF compiler sees empty .instr -> "ISA wrong length"
    mybir.codegen_inst_isa_subclasses(nc)
    _split_multi_waits(nc)
    return nc


def _shard_inputs(input, weight):
    input = np.asarray(input, dtype=np.float32)
    weight = np.asarray(weight, dtype=np.float32)
    in_maps = []
    for n in range(N):
        xp = np.pad(input[n], ((0, 0), (PAD, PAD + 1), (PAD, PAD)))  # [64,133,132]
        for half in range(2):
            r0 = RH * half
            in_maps.append(
                {
                    "xpad": np.ascontiguousarray(xp[:, r0 : r0 + HP, :]),
                    "w": np.ascontiguousarray(
                        weight[n, :, :, r0 * WO : (r0 + RH) * WO]
                    ),
                }
            )
    return in_maps


def kernel(input, weight):
    nc = _build_program(PHASE)
    in_maps = _shard_inputs(input, weight)
    res = run_bass_kernel_spmd(nc, in_maps, list(range(8)))
    out = np.empty((N, C, HO, WO), dtype=np.float32)
    for k in range(8):
        n, half = divmod(k, 2)
        out[n, :, RH * half : RH * (half + 1), :] = res.results[k]["out"]
    return out

